# revision 36
# baseline (speedup 1.0000x reference)
"""Distributed real SHT (spherical harmonic transform) for Trainium2.

Computes, for x [1, 256, 361, 720] f32 and weights [361, 360, 361] f32:
    xf = 2*pi * rfft(x, axis=-1, norm='forward')[..., :361]
    out_re = einsum('bckm,mlk->bclm', Re(xf), weights)
    out_im = einsum('bckm,mlk->bclm', Im(xf), weights)
    return complex64 [1, 256, 360, 361]

Sharding: channels (dim 1) across 8 NeuronCores, 32 channels each.

Fully fused single-pass design (no DRAM scratch for the intermediate):

  Stage A (DFT): the folded input xt is the STATIONARY matmul operand.
    For each tile (s-parity, channel, ke-chunk) of 128 folded-latitude
    columns, psum[ke, (ri, m)] += xt_tile[n, ke]^T @ F[n, (ri, m)] over
    6 slabs (2 DFT branches x 3 contraction slabs of 128 longitudes).
    The PSUM eviction (DVE/ACT alternating) writes bf16 directly into a
    persistent SBUF xf tensor laid out [ke, (c, ri, m)] -- which is
    exactly the layout stage B needs. No transpose, no HBM round trip.

  Stage B (Legendre): 3 matmuls per mode m (the toolchain serializes
    LDWEIGHTS with the matmul stream, so instruction count is what
    matters): two (128-row, 64-col (c,ri)-stationary) matmuls cover
    folded latitudes 0..127 for each l-parity, then ONE parity-merged
    128-row matmul covers latitudes 128..180 -- the xf top tile holds
    both parities in partition bands 0/64 and the doubled wqt has zero
    rows on the opposite band, so a single matmul streams both parity
    column blocks. Waves are interleaved over mp so consecutive matmuls
    alternate PSUM partition bands (hides drain latency; concurrent
    same-bank same-partition drains are a fatal PSUM collision).
    Output stored bf16 (8.4 MB vs 16.8 f32).

Symmetries (exact on the Lobatto grid / integer DFT grid):
  * longitude fold about n=360 (cos/sin branch (anti)symmetry),
  * latitude parity fold P_l^m(-x) = (-1)^(l+m) P_l^m(x).
"""

import numpy as np
import ml_dtypes

BF16 = ml_dtypes.bfloat16

NLAT, NLON = 361, 720
L = 360             # output degrees l = 0..359
MM = 360            # modes used m = 0..359 (m=360 output is all zero)
C = 256
C_LOC = 32
N_CORES = 8
KE = 181            # parity-folded latitudes 0..180
NQ = 90             # quads of 4 modes
NT = 96             # stage-A xt tiles: 64 main + 32 leftover pairs

BATCH = 4
NB = (NQ + BATCH - 1) // BATCH

_QL = [L - 4 * q for q in range(NQ)]
_QLH = [x // 2 for x in _QL]


def _batch_qs(b):
    return range(b * BATCH, min((b + 1) * BATCH, NQ))


_W0_B = [sum(8 * _QLH[q] for q in _batch_qs(b)) for b in range(NB)]
_WT_B = [sum(8 * _QLH[q] for q in _batch_qs(b)) for b in range(NB)]
_OB_B = [sum(4 * _QLH[q] for q in _batch_qs(b)) for b in range(NB)]
_W0_OFF = np.cumsum([0] + _W0_B).tolist()
_WT_OFF = np.cumsum([0] + _WT_B).tolist()
_OB_OFF = np.cumsum([0] + _OB_B).tolist()
W0_TOT = _W0_OFF[-1]      # 65520
WT_TOT = _WT_OFF[-1]      # 65520
OB_TOT = _OB_OFF[-1]      # 32760

_CACHE = {}


def _parity_ls(q, mp):
    """l-lists for quad q, mode m=4q+mp: index p means (l+m) % 2 == p."""
    m = 4 * q + mp
    l0 = 4 * q
    out = []
    for p in range(2):
        start = l0 if (l0 + m) % 2 == p else l0 + 1
        out.append(np.arange(start, L, 2))
    return out


# --------------------------------------------------------------------------
# Bass kernel
# --------------------------------------------------------------------------

def _build_bass(reps=1, stage="AB"):
    import concourse.mybir as mybir
    import concourse.tile as tile
    from concourse import bacc
    from contextlib import nullcontext

    bf16 = mybir.dt.bfloat16
    f32 = mybir.dt.float32

    nc = bacc.Bacc("TRN2", target_bir_lowering=False, debug=False,
                   num_devices=N_CORES)

    xt_d = nc.dram_tensor("xt", [NT * 128, 768], bf16, kind="ExternalInput")
    f_d = nc.dram_tensor("fm", [128, 2 * 3 * MM], bf16, kind="ExternalInput")
    wq0_d = nc.dram_tensor("wq0", [128, W0_TOT], bf16, kind="ExternalInput")
    wqt_d = nc.dram_tensor("wqt", [128, WT_TOT], bf16, kind="ExternalInput")
    ob_d = nc.dram_tensor("ob", [128, OB_TOT], bf16, kind="ExternalOutput")

    with tile.TileContext(nc) as tc:
        with (
            tc.tile_pool(name="fpool", bufs=1) as fpool,
            tc.tile_pool(name="xfp", bufs=1) as xfp,
            tc.tile_pool(name="xtp", bufs=3) as xtp,
            tc.tile_pool(name="wt0p", bufs=2) as wt0p,
            tc.tile_pool(name="wttp", bufs=2) as wttp,
            tc.tile_pool(name="otp", bufs=2) as otp,
            tc.tile_pool(name="psA", bufs=2, space="PSUM") as psA,
            tc.tile_pool(name="psB", bufs=2, space="PSUM") as psB,
            tc.For_i(0, reps, 1) if reps > 1 else nullcontext(),
        ):
            # persistent SBUF xf: [ke, (c 32, ri 2, m 360)] bf16
            xf_m = [xfp.tile([128, C_LOC * 2 * MM], bf16, name=f"xfm{s}",
                             tag=f"xfm{s}") for s in range(2)]
            xf_t = xfp.tile([128, C_LOC * 2 * MM], bf16, name="xft", tag="xft")

            f_tile = fpool.tile([128, 2 * 3 * MM], bf16)
            nc.sync.dma_start(f_tile[:], f_d[:, :])
            fv = f_tile[:].rearrange("p (r a m) -> p r a m", r=2, a=3)

            # ---------------- Stage A: DFT, xt stationary ----------------
            for t in range(NT if stage in ("AB", "A", "B0", "NW", "ND") else 0):
                xt_tile = xtp.tile([128, 768], bf16, tag="xt", bufs=6)
                eng = nc.sync if t % 2 == 0 else nc.scalar
                eng.dma_start(xt_tile[:], xt_d[t * 128:(t + 1) * 128, :])
                xtv = xt_tile[:].rearrange("p (r a w) -> p r a w", r=2, a=3)

                ps = psA.tile([128, 1024], f32, tag="psA")
                psv = ps[:].rearrange("p (r x) -> p r x", r=2)
                for par in range(2):
                    for a in range(3):
                        nc.tensor.matmul(
                            psv[:, par, 0:MM],
                            xtv[:, par, a, :],
                            fv[:, par, a, :],
                            start=(a == 0),
                            stop=(a == 2),
                        )
                if t < 64:
                    s, c = t // 32, t % 32
                    dst_tile = xf_m[s]
                else:
                    c = t - 64
                    dst_tile = xf_t
                dst = dst_tile[:, c * 720:(c + 1) * 720].rearrange(
                    "p (r m) -> p r m", r=2)
                if t % 2 == 0:
                    nc.vector.tensor_copy(dst, psv[:, :, 0:MM])
                else:
                    nc.scalar.copy(dst, psv[:, :, 0:MM])

            # xf views for stage B: [ke, c, ri, m] -- (c, ri) col order gives
            # a single-stride (360) lhsT free dim, as the matmul AP requires.
            xfv_m = [xf_m[s][:].rearrange("k (c r m) -> k c r m",
                                          c=C_LOC, r=2) for s in range(2)]
            xfv_t = xf_t[:].rearrange("k (c r m) -> k c r m", c=C_LOC, r=2)

            # ---------------- Stage B: Legendre contraction ----------------
            wt_once = [None, None]
            for b in range(NB if stage in ("AB", "B", "B0", "NW", "ND")
                           else 0):
                w0w, wtw, obw = _W0_B[b], _WT_B[b], _OB_B[b]
                if stage == "NW" and wt_once[0] is not None:
                    wt0, wtt = wt_once     # timing probe: reuse batch-0 wts
                else:
                    e0 = nc.sync if b % 2 == 0 else nc.scalar
                    e1 = nc.scalar if b % 2 == 0 else nc.sync
                    wt0 = wt0p.tile([128, w0w], bf16, tag="wt0")
                    e0.dma_start(
                        wt0[:], wq0_d[:, _W0_OFF[b]:_W0_OFF[b] + w0w])
                    wtt = wttp.tile([128, wtw], bf16, tag="wtt")
                    e1.dma_start(
                        wtt[:], wqt_d[:, _WT_OFF[b]:_WT_OFF[b] + wtw])
                    wt_once = [wt0, wtt]
                ot = otp.tile([128, obw], bf16, tag="ot")
                if stage == "ND":
                    nc.vector.memset(ot[:], 0.0)
                    nc.sync.dma_start(
                        ob_d[:, _OB_OFF[b]:_OB_OFF[b] + obw], ot[:])
                    continue

                o0 = ott = 0
                for q in _batch_qs(b):
                    Lh = _QLH[q]
                    ps = psB.tile([128, 1024], f32, tag="psB")
                    # 3 matmuls per mode: ck0 per parity (128-row, 64-col
                    # (c,ri) stationary), then ONE parity-merged ck1 (the
                    # xf top tile holds both parities in partition bands;
                    # the doubled wtt has zero rows on the other band so a
                    # full-128-row matmul streams both parity col-blocks).
                    # Waves interleave mp so consecutive matmuls alternate
                    # PSUM partition bands (hides the drain latency).
                    for wave in range(3):
                        for mp in range(4):
                            m = 4 * q + mp
                            band, blk = mp % 2, mp // 2
                            if wave < 2:
                                p = wave
                                out = ps[band * 64:(band + 1) * 64,
                                         blk * 512 + p * Lh:
                                         blk * 512 + (p + 1) * Lh]
                                lh = xfv_m[p][:, :, :, m]
                                rh = wt0[:, o0 + (p * 4 + mp) * Lh:
                                         o0 + (p * 4 + mp + 1) * Lh]
                            else:
                                out = ps[band * 64:(band + 1) * 64,
                                         blk * 512:blk * 512 + 2 * Lh]
                                lh = xfv_t[:, :, :, m]
                                rh = wtt[:, ott + 2 * mp * Lh:
                                         ott + 2 * (mp + 1) * Lh]
                            if stage == "B0" and wave == 2:
                                continue
                            nc.tensor.matmul(
                                out, lh, rh,
                                start=(wave == 0),
                                stop=(wave == 2 or
                                      (stage == "B0" and wave == 1)),
                            )
                    src = ps[:].rearrange(
                        "p (u x) -> p u x", u=2)[:, :, 0:2 * Lh]
                    dstv = ot[:, (ott // 2):(ott // 2) + 4 * Lh].rearrange(
                        "p (u x) -> p u x", u=2)
                    if q % 2 == 0:
                        nc.vector.tensor_copy(dstv, src)
                    else:
                        nc.scalar.copy(dstv, src)
                    o0 += 8 * Lh
                    ott += 8 * Lh
                nc.gpsimd.dma_start(
                    ob_d[:, _OB_OFF[b]:_OB_OFF[b] + obw], ot[:])

    nc.compile()
    return nc


# --------------------------------------------------------------------------
# Host-side packing (validated against reference in sim.py)
# --------------------------------------------------------------------------

def _pack_xt(x_core):
    """x_core [32, 361, 720] f32 -> xt tiles [96*128, 768] bf16."""
    xn = x_core.transpose(2, 1, 0)            # [720 n, 361 k, 32 c]
    xkf = np.zeros((NLON, 2, KE, C_LOC), np.float32)
    for s in range(2):
        sign = 1.0 - 2.0 * s
        xkf[:, s, :180] = xn[:, :180] + sign * xn[:, NLAT - 1:180:-1]
        xkf[:, s, 180] = xn[:, 180]
    nh = NLON // 2
    xt = np.zeros((2, 384, 2, KE, C_LOC), np.float32)
    xt[0, 0] = xkf[0]
    xt[0, 1:nh] = xkf[1:nh] + xkf[:nh:-1]
    xt[0, nh] = xkf[nh]
    xt[1, 1:nh] = xkf[1:nh] - xkf[:nh:-1]

    tiles = np.zeros((NT, 128, 2, 3, 128), np.float32)
    for t in range(64):
        s, c = t // 32, t % 32
        for a in range(3):
            tiles[t, :, :, a, :] = xt[:, a * 128:(a + 1) * 128, s, :128, c]\
                .transpose(1, 0, 2)
    for t in range(32):
        c = t
        for a in range(3):
            blk = xt[:, a * 128:(a + 1) * 128, :, 128:181, c]
            tiles[64 + t, :, :, a, 0:53] = blk[:, :, 0, :].transpose(1, 0, 2)
            tiles[64 + t, :, :, a, 64:117] = blk[:, :, 1, :].transpose(1, 0, 2)
    return tiles.reshape(NT * 128, 768).astype(BF16)


def _pack_f():
    n = np.arange(NLAT, dtype=np.float64)[:, None]
    m = np.arange(MM, dtype=np.float64)[None, :]
    ang = 2.0 * np.pi * n * m / NLON
    coef = 2.0 * np.pi / NLON
    F = np.zeros((2, 384, MM), np.float32)
    F[0, :NLAT] = (coef * np.cos(ang)).astype(np.float32)
    F[1, :NLAT] = (-coef * np.sin(ang)).astype(np.float32)
    out = np.zeros((128, 2, 3, MM), np.float32)
    for par in range(2):
        for a in range(3):
            out[:, par, a, :] = F[par, a * 128:(a + 1) * 128, :]
    return out.reshape(128, 2 * 3 * MM).astype(BF16)


def _pack_wq(weights):
    rev = np.arange(NLAT - 1, -1, -1)
    wq0 = np.zeros((128, W0_TOT), np.float32)
    wqt = np.zeros((128, WT_TOT), np.float32)
    for b in range(NB):
        o0 = _W0_OFF[b]
        ot = _WT_OFF[b]
        for q in _batch_qs(b):
            Lh = _QLH[q]
            for mp in range(4):
                m = 4 * q + mp
                lls = _parity_ls(q, mp)
                for p in range(2):
                    ls = lls[p]
                    sign = 1.0 - 2.0 * p
                    Wm = weights[m][ls]                      # [Lh, 361]
                    W = np.zeros((len(ls), KE), np.float32)
                    W[:, :180] = 0.5 * (Wm[:, :180] + sign * Wm[:, rev[:180]])
                    W[:, 180] = Wm[:, 180]
                    wq0[:, o0 + (p * 4 + mp) * Lh:o0 + (p * 4 + mp + 1) * Lh]\
                        = W[:, :128].T
                    # parity-merged top chunk: cols (mp, p, Lh); rows of the
                    # other parity band stay zero so one 128-row matmul
                    # serves both parities.
                    ct = ot + 2 * mp * Lh + p * Lh
                    wqt[64 * p:64 * p + 53, ct:ct + Lh] = W[:, 128:181].T
            o0 += 8 * Lh
            ot += 8 * Lh
    return wq0.astype(BF16), wqt.astype(BF16)


# --------------------------------------------------------------------------
# Runner: jit(shard_map(bass_exec)) over the 8 cores
# --------------------------------------------------------------------------

class _Runner:
    def __init__(self, nc):
        import jax
        import concourse.mybir as mybir
        from jax.experimental.shard_map import shard_map
        from jax.sharding import Mesh, PartitionSpec, NamedSharding
        from concourse.bass2jax import (
            _bass_exec_p, install_neuronx_cc_hook, partition_id_tensor)

        install_neuronx_cc_hook()
        self.jax = jax
        self.nc = nc
        part_name = (nc.partition_id_tensor.name
                     if nc.partition_id_tensor else None)
        in_names, out_names, out_avals, zero_outs = [], [], [], []
        for alloc in nc.m.functions[0].allocations:
            if not isinstance(alloc, mybir.MemoryLocationSet):
                continue
            name = alloc.memorylocations[0].name
            if alloc.kind == "ExternalInput":
                if name != part_name:
                    in_names.append(name)
            elif alloc.kind == "ExternalOutput":
                shape = tuple(alloc.tensor_shape)
                dtype = mybir.dt.np(alloc.dtype)
                out_names.append(name)
                out_avals.append(jax.core.ShapedArray(shape, dtype))
                zero_outs.append(np.zeros(shape, dtype))
        self.in_names = list(in_names)
        self.out_names = out_names
        self.out_avals = out_avals
        self.zero_outs = zero_outs
        all_names = in_names + out_names
        if part_name is not None:
            all_names = all_names + [part_name]

        def _body(*args):
            operands = list(args)
            if part_name is not None:
                operands.append(partition_id_tensor())
            outs = _bass_exec_p.bind(
                *operands,
                out_avals=tuple(out_avals),
                in_names=tuple(all_names),
                out_names=tuple(out_names),
                lowering_input_output_aliases=(),
                sim_require_finite=True,
                sim_require_nnan=True,
                nc=nc,
            )
            return tuple(outs)

        devices = jax.devices()[:N_CORES]
        mesh = Mesh(np.asarray(devices), ("core",))
        spec = PartitionSpec("core")
        n_args = len(in_names) + len(out_names)
        self.sharding = NamedSharding(mesh, spec)
        self.fn = jax.jit(
            shard_map(_body, mesh=mesh,
                      in_specs=(spec,) * n_args,
                      out_specs=(spec,) * len(out_names),
                      check_rep=False),
            keep_unused=True,
        )

    def device_args(self, in_maps):
        jax = self.jax
        args = []
        for name in self.in_names:
            cat = np.concatenate([m[name] for m in in_maps], axis=0)
            args.append(jax.device_put(cat, self.sharding))
        for z in self.zero_outs:
            cat = np.zeros((N_CORES * z.shape[0], *z.shape[1:]), z.dtype)
            args.append(jax.device_put(cat, self.sharding))
        return args

    def execute(self, args):
        outs = self.fn(*args)
        self.jax.block_until_ready(outs)
        return outs

    def run(self, in_maps):
        outs = self.execute(self.device_args(in_maps))
        results = []
        for c in range(N_CORES):
            r = {}
            for i, name in enumerate(self.out_names):
                full = np.asarray(outs[i])
                r[name] = full.reshape(N_CORES, *self.out_avals[i].shape)[c]
            results.append(r)
        return results


def get_runner(reps=1, stage="AB"):
    key = ("runner", reps, stage)
    if key not in _CACHE:
        _CACHE[key] = _Runner(_build_bass(reps, stage))
    return _CACHE[key]


def prepare_in_maps(x, weights):
    x = np.asarray(x, dtype=np.float32)
    weights = np.asarray(weights, dtype=np.float32)
    if "F" not in _CACHE:
        _CACHE["F"] = _pack_f()
    F = _CACHE["F"]
    wq0, wqt = _pack_wq(weights)
    in_maps = []
    for pc in range(N_CORES):
        xt = _pack_xt(x[0, pc * C_LOC:(pc + 1) * C_LOC])
        in_maps.append({"xt": xt, "fm": F, "wq0": wq0, "wqt": wqt})
    return in_maps


def unpack_results(results):
    out_re = np.zeros((C, L, NLAT), np.float32)
    out_im = np.zeros((C, L, NLAT), np.float32)
    qoffs = np.cumsum([0] + [4 * x for x in _QLH]).tolist()
    for pc in range(N_CORES):
        ob = np.asarray(results[pc]["ob"], dtype=np.float32)
        c0 = pc * C_LOC
        for q in range(NQ):
            Lh = _QLH[q]
            blob = ob[:, qoffs[q]:qoffs[q + 1]]
            # partitions (band, c, ri); cols (blk, p, l')
            blob = blob.reshape(2, 32, 2, 2, 2, Lh)
            for mp in range(4):
                m = 4 * q + mp
                band, blk = mp % 2, mp // 2
                lls = _parity_ls(q, mp)
                for p in range(2):
                    ls = lls[p]
                    out_re[c0:c0 + 32, ls, m] = \
                        blob[band, :, 0, blk, p, :len(ls)]
                    out_im[c0:c0 + 32, ls, m] = \
                        blob[band, :, 1, blk, p, :len(ls)]
    out = (out_re + 1j * out_im).astype(np.complex64)
    return out.reshape(1, C, L, NLAT)


def kernel(x, weights):
    runner = get_runner()
    in_maps = prepare_in_maps(x, weights)
    results = runner.run(in_maps)
    return unpack_results(results)
